# revision 1
# baseline (speedup 1.0000x reference)
"""Trainium2 Bass kernel for the DialogGCN GAT-style message-passing layer.

Math notes (why this is much cheaper than the reference graph):
  Kp    = concat(K, kfeat) @ Wk + bk                    (B,N,D)
  alpha = Q@wden[:D] + Kp@wden[D:] + bden               (B,N)
  w     = softmax(alpha - (1-adj)*1e30, axis=N)
  out   = sum_n w * ((Kp@Wr0)*sm + (Kp@Wr1)*(1-sm))

* softmax is invariant to per-row constants, so the Q term, bden and the
  bk@wden[D:] constant all cancel:  w = softmax_n(X_n . v) masked, where
  X = concat(K, kfeat) and v = Wk @ wden[D:]  (folded on host).
* the output is linear in the weighted sums:
    out = (sum_n w*sm*X_n | c0) @ [Wk;bk] @ Wr0 + (sum_n w*(1-sm)*X_n | c1) @ [Wk;bk] @ Wr1
  so G0 = [Wk;bk]@Wr0 and G1 = [Wk;bk]@Wr1 are folded on host (769x512 each)
  and the device only needs one streaming pass over X computing
    s_n = X_n . v ; p_n = exp(s_n)*adj_n ; U0 = sum p*sm*[X|1] ; U1 = sum p*(1-sm)*[X|1]
  followed by a tiny projection (U0@G0 + U1@G1) / P.

Sharding: pure data parallel over batch B=32 across 8 cores (4 rows each).
"""

import os
import sys

import numpy as np

for _p in ("/opt/trn_rl_repo", "/root/.axon_site/_ro/trn_rl_repo"):
    if os.path.isdir(_p) and _p not in sys.path:
        sys.path.insert(0, _p)

B, N, D, KD = 32, 2048, 512, 256
F = D + KD  # 768
NCORES = 8
BL = B // NCORES  # 4 batch rows per core
NT = 16  # free-dim token tiles per batch (N = 128 * NT)

_BUILD_CACHE = {}
last_results = None  # BassKernelResults of the most recent run (for test.py)


def _build(stream_f32r: bool):
    """Trace the Bass program (same NEFF runs SPMD on all 8 cores)."""
    import concourse.bass as bass
    import concourse.tile as tile
    from concourse import bacc, mybir
    from concourse.masks import make_identity

    f32 = mybir.dt.float32
    i32 = mybir.dt.int32
    mm_dt = mybir.dt.float32r if stream_f32r else f32

    nc = bacc.Bacc()

    # ---- DRAM I/O ----------------------------------------------------------
    # X inputs carry the streaming-matmul dtype (float32r == float32 bits;
    # only the PE interprets it as tf32) so the BIR verifier sees consistent
    # producer/consumer dtypes on the fp32r path.
    xK_f = nc.dram_tensor("xK_f", [BL, N, D], mm_dt, kind="ExternalInput")
    xk1_f = nc.dram_tensor("xk1_f", [BL, N, KD], mm_dt, kind="ExternalInput")
    xK_b = nc.dram_tensor("xK_b", [BL, N, D], mm_dt, kind="ExternalInput")
    xk1_b = nc.dram_tensor("xk1_b", [BL, N, KD], mm_dt, kind="ExternalInput")
    adj_f = nc.dram_tensor("adj_f", [BL, N], i32, kind="ExternalInput")
    sm_f = nc.dram_tensor("sm_f", [BL, N], i32, kind="ExternalInput")
    adj_b = nc.dram_tensor("adj_b", [BL, N], i32, kind="ExternalInput")
    sm_b = nc.dram_tensor("sm_b", [BL, N], i32, kind="ExternalInput")
    v_f = nc.dram_tensor("v_f", [F], f32, kind="ExternalInput")
    v_b = nc.dram_tensor("v_b", [F], f32, kind="ExternalInput")
    G0_f = nc.dram_tensor("G0_f", [F + 1, D], f32, kind="ExternalInput")
    G1_f = nc.dram_tensor("G1_f", [F + 1, D], f32, kind="ExternalInput")
    G0_b = nc.dram_tensor("G0_b", [F + 1, D], f32, kind="ExternalInput")
    G1_b = nc.dram_tensor("G1_b", [F + 1, D], f32, kind="ExternalInput")
    out_f = nc.dram_tensor("out_f", [BL, D], f32, kind="ExternalOutput")
    out_b = nc.dram_tensor("out_b", [BL, D], f32, kind="ExternalOutput")

    branches = [
        dict(xK=xK_f, xk1=xk1_f, adj=adj_f, sm=sm_f, v=v_f, G0=G0_f, G1=G1_f, out=out_f),
        dict(xK=xK_b, xk1=xk1_b, adj=adj_b, sm=sm_b, v=v_b, G0=G0_b, G1=G1_b, out=out_b),
    ]

    with tile.TileContext(nc) as tc:
        with (
            tc.tile_pool(name="singles", bufs=1) as singles,
            tc.tile_pool(name="xKp", bufs=2) as xKp,
            tc.tile_pool(name="xk1p", bufs=3) as xk1p,
            tc.tile_pool(name="scr", bufs=3) as scr,
            tc.tile_pool(name="small", bufs=4) as small,
            tc.tile_pool(name="uallp", bufs=2) as uallp,
            tc.tile_pool(name="uallTp", bufs=2) as uallTp,
            tc.tile_pool(name="finp", bufs=2) as finp,
            tc.tile_pool(name="psU_K", bufs=2, space="PSUM") as psU_K,
            tc.tile_pool(name="psU_1", bufs=2, space="PSUM") as psU_1,
            tc.tile_pool(name="psTr", bufs=2, space="PSUM") as psTr,
            tc.tile_pool(name="psOut", bufs=1, space="PSUM") as psOut,
        ):
            # ---- one-time setup -------------------------------------------
            ident = singles.tile([128, 128], f32)
            make_identity(nc, ident)
            ones11 = singles.tile([1, 1], f32)
            nc.vector.memset(ones11, 1.0)
            # f32 zeros used to produce f32r-typed zeros/ones (memset can't
            # write f32r, but tensor ops can)
            zf = singles.tile([128, NT, 8], f32)
            nc.vector.memset(zf, 0.0)
            # (128,2) ones in the matmul dtype: moving operand of the
            # softmax-denominator accumulation
            ones2 = singles.tile([128, 2], mm_dt)
            nc.vector.tensor_scalar_add(out=ones2, in0=zf[:, 0, 0:2], scalar1=1.0)

            per_br = []
            for br in branches:
                st = {}
                # score vector broadcast across partitions: (128, 768)
                vb = singles.tile([128, F], f32)
                vap = br["v"][:]
                nc.gpsimd.dma_start(
                    out=vb,
                    in_=bass.AP(tensor=vap.tensor, offset=vap.offset, ap=[[0, 128]] + vap.ap),
                )
                st["vb"] = vb
                # G matrices: (128, 7, 512); chunk 6 row 0 holds row 768
                for gname in ("G0", "G1"):
                    g = br[gname]
                    gs = singles.tile([128, 7, D], f32)
                    nc.gpsimd.dma_start(
                        out=gs[:, 0:6, :],
                        in_=g[0:F, :].rearrange("(k p) n -> p k n", p=128),
                    )
                    nc.gpsimd.dma_start(out=gs[0:1, 6, :], in_=g[F : F + 1, :])
                    st[gname] = gs
                # masks for all BL batches: (128, BL, NT), token = p*NT + n
                adj_i = small.tile([128, BL, NT], i32, tag="mask_i")
                sm_i = small.tile([128, BL, NT], i32, tag="mask_i")
                nc.gpsimd.dma_start(out=adj_i, in_=br["adj"].rearrange("b (p n) -> p b n", n=NT))
                nc.gpsimd.dma_start(out=sm_i, in_=br["sm"].rearrange("b (p n) -> p b n", n=NT))
                adjf = small.tile([128, BL, NT], f32, tag="mask_f")
                smf = small.tile([128, BL, NT], f32, tag="mask_f")
                nc.vector.tensor_copy(adjf, adj_i)
                nc.vector.tensor_copy(smf, sm_i)
                m0 = singles.tile([128, BL, NT], f32, tag=f"m0_{br['out'].name}")
                m1 = singles.tile([128, BL, NT], f32, tag=f"m1_{br['out'].name}")
                nc.vector.tensor_mul(m0, adjf, smf)
                nc.vector.tensor_sub(m1, adjf, m0)
                st["m0"], st["m1"] = m0, m1
                per_br.append(st)

            # ---- streaming + finishing per branch -------------------------
            for bi, br in enumerate(branches):
                st = per_br[bi]
                psK = psU_K.tile([8, D], f32)       # rows 0-3: U0(b), rows 4-7: U1(b)
                # cols 0:KD = U_k1, col KD = ones column (P0/P1), col KD+1 = pad
                # (fp32r matmul needs even moving-free-size / 8B alignment)
                ps1 = psU_1.tile([8, KD + 2], f32)

                for b in range(BL):
                    # contiguous-destination tiles keep SWDGE descriptor
                    # generation cheap (strided dst was costing ~13us/unit on Q7)
                    xK = xKp.tile([128, NT, D], mm_dt, tag="xK")
                    nc.gpsimd.dma_start(
                        out=xK, in_=br["xK"][b].rearrange("(p n) d -> p n d", n=NT)
                    )
                    xk1 = xk1p.tile([128, NT, KD], mm_dt, tag="xk1")
                    nc.gpsimd.dma_start(
                        out=xk1, in_=br["xk1"][b].rearrange("(p n) d -> p n d", n=NT)
                    )
                    xK_f32 = xK[:, :, :].bitcast(f32)
                    xk1_f32 = xk1[:, :, :].bitcast(f32)

                    sA = small.tile([128, NT], f32, tag="sA")
                    sB = small.tile([128, NT], f32, tag="sB")
                    prodK = scr.tile([128, D], f32, tag="prodK")
                    prod1 = scr.tile([128, KD], f32, tag="prod1")
                    for n in range(NT):
                        nc.vector.scalar_tensor_tensor(
                            out=prodK,
                            in0=xK_f32[:, n, :],
                            scalar=0.0,
                            in1=st["vb"][:, 0:D],
                            op0=mybir.AluOpType.bypass,
                            op1=mybir.AluOpType.mult,
                            accum_out=sA[:, n : n + 1],
                        )
                        nc.vector.scalar_tensor_tensor(
                            out=prod1,
                            in0=xk1_f32[:, n, :],
                            scalar=0.0,
                            in1=st["vb"][:, D:F],
                            op0=mybir.AluOpType.bypass,
                            op1=mybir.AluOpType.mult,
                            accum_out=sB[:, n : n + 1],
                        )
                    nc.vector.tensor_add(sB, sA, sB)
                    p_raw = small.tile([128, NT], f32, tag="p_raw")
                    nc.scalar.activation(out=p_raw, in_=sB, func=mybir.ActivationFunctionType.Exp)

                    # pp[:, n, :]: col b = p*adj*sm, col 4+b = p*adj*(1-sm), rest 0
                    pp = small.tile([128, NT, 8], mm_dt, tag="pp")
                    nc.vector.tensor_mul(pp, zf, zf)
                    nc.vector.tensor_mul(pp[:, :, b], p_raw, st["m0"][:, b, :])
                    nc.vector.tensor_mul(pp[:, :, 4 + b], p_raw, st["m1"][:, b, :])

                    for n in range(NT):
                        first = b == 0 and n == 0
                        last = b == BL - 1 and n == NT - 1
                        nc.tensor.matmul(psK, pp[:, n, :], xK[:, n, :], start=first, stop=last)
                        # k1 accumulate + softmax-denominator ones column share
                        # one PSUM group (partial-width writes accumulate fine)
                        nc.tensor.matmul(
                            ps1[:, 0:KD], pp[:, n, :], xk1[:, n, :], start=first, stop=False
                        )
                        nc.tensor.matmul(
                            ps1[:, KD : KD + 2],
                            pp[:, n, :],
                            ones2,
                            start=False,
                            stop=last,
                        )

                # ---- finishing: out = (U0@G0 + U1@G1) / P ------------------
                uall = uallp.tile([8, F + 1], f32)
                nc.vector.tensor_copy(uall[:, 0:D], psK)
                nc.vector.tensor_copy(uall[:, D : F + 1], ps1[:, 0 : KD + 1])

                uallT = uallTp.tile([128, 7, 8], f32)
                for k in range(6):
                    trp = psTr.tile([128, 8], f32)
                    nc.tensor.transpose(trp, uall[:, k * 128 : (k + 1) * 128], ident[0:8, 0:8])
                    nc.vector.tensor_copy(uallT[:, k, :], trp)
                trp = psTr.tile([128, 8], f32)
                nc.tensor.transpose(trp[0:1, :], uall[:, F : F + 1], ident[0:8, 0:8])
                nc.vector.tensor_copy(uallT[0:1, 6, :], trp[0:1, :])

                po = psOut.tile([4, D + 1], f32)  # cols 0:D main, col D = P (bank 2)
                for k in range(6):
                    nc.tensor.matmul(
                        po[:, 0:D], uallT[:, k, 0:4], st["G0"][:, k, :], start=(k == 0), stop=False
                    )
                nc.tensor.matmul(
                    po[:, 0:D], uallT[0:1, 6, 0:4], st["G0"][0:1, 6, :], start=False, stop=False
                )
                for k in range(6):
                    nc.tensor.matmul(
                        po[:, 0:D], uallT[:, k, 4:8], st["G1"][:, k, :], start=False, stop=False
                    )
                nc.tensor.matmul(
                    po[:, 0:D], uallT[0:1, 6, 4:8], st["G1"][0:1, 6, :], start=False, stop=True
                )
                nc.tensor.matmul(po[:, D : D + 1], uallT[0:1, 6, 0:4], ones11, start=True, stop=False)
                nc.tensor.matmul(po[:, D : D + 1], uallT[0:1, 6, 4:8], ones11, start=False, stop=True)

                rp = finp.tile([4, 1], f32, tag="rp")
                nc.vector.reciprocal(rp, po[:, D : D + 1])
                osb = finp.tile([4, D], f32, tag="osb")
                nc.vector.tensor_scalar_mul(out=osb, in0=po[:, 0:D], scalar1=rp)
                nc.sync.dma_start(out=br["out"][:, :], in_=osb)

    nc.compile()
    return nc


def _get_nc(stream_f32r: bool):
    key = ("nc", stream_f32r)
    if key not in _BUILD_CACHE:
        _BUILD_CACHE[key] = _build(stream_f32r)
    return _BUILD_CACHE[key]


def kernel(**inputs) -> tuple:
    global last_results
    from concourse.bass_utils import run_bass_kernel_spmd

    f32 = np.float32
    K = np.ascontiguousarray(np.asarray(inputs["K"], dtype=f32))
    front_k1 = np.ascontiguousarray(np.asarray(inputs["front_k1"], dtype=f32))
    back_K = np.ascontiguousarray(np.asarray(inputs["back_K"], dtype=f32))
    back_k2 = np.ascontiguousarray(np.asarray(inputs["back_k2"], dtype=f32))
    Wfk = np.asarray(inputs["Wfk"], dtype=f32)
    bfk = np.asarray(inputs["bfk"], dtype=f32)
    Wbk = np.asarray(inputs["Wbk"], dtype=f32)
    bbk = np.asarray(inputs["bbk"], dtype=f32)
    Wr0 = np.asarray(inputs["Wr0"], dtype=f32)
    Wr1 = np.asarray(inputs["Wr1"], dtype=f32)
    wf_den = np.asarray(inputs["wf_den"], dtype=f32)
    wb_den = np.asarray(inputs["wb_den"], dtype=f32)
    adj_f = np.ascontiguousarray(np.asarray(inputs["front_sdj_den"], dtype=np.int32))
    sm_f = np.ascontiguousarray(np.asarray(inputs["front_s_mask"], dtype=np.int32))
    adj_b = np.ascontiguousarray(np.asarray(inputs["back_sdj_den"], dtype=np.int32))
    sm_b = np.ascontiguousarray(np.asarray(inputs["back_s_mask"], dtype=np.int32))
    i = int(np.asarray(inputs["i"]))
    num_utter = int(np.asarray(inputs["num_utter"]))

    # host-folded weights
    v_f = (Wfk.astype(np.float64) @ wf_den[D:].astype(np.float64)).astype(f32)
    v_b = (Wbk.astype(np.float64) @ wb_den[D:].astype(np.float64)).astype(f32)
    A_f = np.vstack([Wfk, bfk[None, :]]).astype(np.float64)
    A_b = np.vstack([Wbk, bbk[None, :]]).astype(np.float64)
    G0_f = (A_f @ Wr0.astype(np.float64)).astype(f32)
    G1_f = (A_f @ Wr1.astype(np.float64)).astype(f32)
    G0_b = (A_b @ Wr0.astype(np.float64)).astype(f32)
    G1_b = (A_b @ Wr1.astype(np.float64)).astype(f32)

    stream_f32r = os.environ.get("KERNEL_MM_F32R", "1") == "1"
    nc = _get_nc(stream_f32r)

    in_maps = []
    for c in range(NCORES):
        s = slice(c * BL, (c + 1) * BL)
        in_maps.append(
            {
                "xK_f": K[s],
                "xk1_f": front_k1[s],
                "xK_b": back_K[s],
                "xk1_b": back_k2[s],
                "adj_f": adj_f[s],
                "sm_f": sm_f[s],
                "adj_b": adj_b[s],
                "sm_b": sm_b[s],
                "v_f": v_f,
                "v_b": v_b,
                "G0_f": G0_f,
                "G1_f": G1_f,
                "G0_b": G0_b,
                "G1_b": G1_b,
            }
        )

    trace = os.environ.get("KERNEL_TRACE", "0") == "1"
    res = run_bass_kernel_spmd(nc, in_maps, core_ids=list(range(NCORES)), trace=trace)
    last_results = res

    front = np.concatenate([r["out_f"] for r in res.results], axis=0)
    back = np.concatenate([r["out_b"] for r in res.results], axis=0)
    if i == 0:
        front = np.zeros((B, D), dtype=f32)
    if i == num_utter - 1:
        back = np.zeros((B, D), dtype=f32)
    return (front, back)



# revision 2
# speedup vs baseline: 1.4063x; 1.4063x over previous
"""Trainium2 Bass kernel for the DialogGCN GAT-style message-passing layer.

Math notes (why this is much cheaper than the reference graph):
  Kp    = concat(K, kfeat) @ Wk + bk                    (B,N,D)
  alpha = Q@wden[:D] + Kp@wden[D:] + bden               (B,N)
  w     = softmax(alpha - (1-adj)*1e30, axis=N)
  out   = sum_n w * ((Kp@Wr0)*sm + (Kp@Wr1)*(1-sm))

* softmax is invariant to per-row constants, so the Q term, bden and the
  bk@wden[D:] constant all cancel:  w = softmax_n(X_n . v) masked, where
  X = concat(K, kfeat) and v = Wk @ wden[D:]  (folded on host).
* the output is linear in the weighted sums:
    out = (sum_n w*sm*X_n | c0) @ [Wk;bk] @ Wr0 + (sum_n w*(1-sm)*X_n | c1) @ [Wk;bk] @ Wr1
  so G0 = [Wk;bk]@Wr0 and G1 = [Wk;bk]@Wr1 are folded on host (769x512 each)
  and the device only needs one streaming pass over X computing
    s_n = X_n . v ; p_n = exp(s_n) ; U0 = sum p*m0*[X|1] ; U1 = sum p*m1*[X|1]
  followed by a tiny projection (U0@G0 + U1@G1) / P, with m0 = adj*sm,
  m1 = adj*(1-sm), P = row 768 of U0+U1 (the ones column of X).

Device-side layout tricks:
* X is uploaded as ONE bf16 tensor [BL, N, 772] = [K | k1 | 1.0 | 0 0 0]
  (772 keeps every 128-token chunk 4B/8B aligned). This halves HBM traffic,
  lets the DVE run the score dot-products in 2x packed mode, and the ones
  column makes the softmax denominator fall out of the same PE accumulation
  that computes U (no extra per-chunk matmuls).
* masks, the score vector v (broadcast to 128 partitions) and the G
  projection matrices are pre-swizzled to their SBUF layouts on host and
  uploaded bf16, so every DMA is a dtype-preserving HWDGE transfer with
  contiguous per-partition descriptors.

Sharding: pure data parallel over batch B=32 across 8 cores (4 rows each).
"""

import os
import sys

import numpy as np

for _p in ("/opt/trn_rl_repo", "/root/.axon_site/_ro/trn_rl_repo"):
    if os.path.isdir(_p) and _p not in sys.path:
        sys.path.insert(0, _p)

B, N, D, KD = 32, 2048, 512, 256
F = D + KD  # 768
XW = F + 4  # 772: [K | k1 | 1 | 0 0 0] -- pad keeps chunk offsets 8B aligned
NCORES = 8
BL = B // NCORES  # 4 batch rows per core
NT = 16  # free-dim token tiles per batch (N = 128 * NT)

_BUILD_CACHE = {}
last_results = None  # BassKernelResults of the most recent run (for test.py)


def _build():
    """Trace the Bass program (same NEFF runs SPMD on all 8 cores)."""
    import concourse.bass as bass
    import concourse.tile as tile
    from concourse import bacc, mybir
    from concourse.masks import make_identity

    f32 = mybir.dt.float32
    bf16 = mybir.dt.bfloat16

    nc = bacc.Bacc()

    # ---- DRAM I/O ----------------------------------------------------------
    x_f = nc.dram_tensor("x_f", [BL, N, XW], bf16, kind="ExternalInput")
    x_b = nc.dram_tensor("x_b", [BL, N, XW], bf16, kind="ExternalInput")
    # v broadcast to all partitions, per branch
    vbm = nc.dram_tensor("vbm", [128, 2, XW], bf16, kind="ExternalInput")
    # masks pre-swizzled: [partition, branch, m0/m1, batch, n]
    mks = nc.dram_tensor("mks", [128, 2, 2, BL, NT], bf16, kind="ExternalInput")
    # G matrices pre-swizzled: [partition, (G0_f,G1_f,G0_b,G1_b), chunk, D]
    gpk = nc.dram_tensor("gpk", [128, 4, 7, D], bf16, kind="ExternalInput")
    out_f = nc.dram_tensor("out_f", [BL, D], f32, kind="ExternalOutput")
    out_b = nc.dram_tensor("out_b", [BL, D], f32, kind="ExternalOutput")

    with tile.TileContext(nc) as tc:
        with (
            tc.tile_pool(name="singles", bufs=1) as singles,
            tc.tile_pool(name="xp", bufs=4) as xp,
            tc.tile_pool(name="scp", bufs=2) as scp,
            tc.tile_pool(name="prodp", bufs=2) as prodp,
            tc.tile_pool(name="ppp", bufs=3) as ppp,
            tc.tile_pool(name="finp", bufs=2) as finp,
            tc.tile_pool(name="psA", bufs=2, space="PSUM") as psA,
            tc.tile_pool(name="psB", bufs=2, space="PSUM") as psB,
            tc.tile_pool(name="psTr", bufs=2, space="PSUM") as psTr,
            tc.tile_pool(name="psOut", bufs=1, space="PSUM") as psOut,
        ):
            # ---- one-time setup -------------------------------------------
            ident = singles.tile([8, 8], f32)
            make_identity(nc, ident)
            ones11 = singles.tile([1, 1], bf16)
            nc.vector.memset(ones11, 1.0)

            vbt = singles.tile([128, 2, XW], bf16)
            nc.scalar.dma_start(out=vbt, in_=vbm[:, :, :])
            mkt = singles.tile([128, 2, 2, BL, NT], bf16)
            nc.scalar.dma_start(out=mkt, in_=mks[:, :, :, :, :])
            gt = singles.tile([128, 4, 7, D], bf16)
            nc.scalar.dma_start(out=gt, in_=gpk[:, :, :, :])

            # ---- streaming + finishing per branch -------------------------
            for ibr, (xsrc, osrc) in enumerate(((x_f, out_f), (x_b, out_b))):
                psAt = psA.tile([8, D], f32)       # rows 0-3: U0(b) K-part, 4-7: U1(b)
                psBt = psB.tile([8, KD + 4], f32)  # cols 0:KD k1-part, col KD = P, pad

                for b in range(BL):
                    x = xp.tile([128, NT, XW], bf16, tag="x")
                    nc.sync.dma_start(
                        out=x, in_=xsrc[b].rearrange("(p n) d -> p n d", n=NT)
                    )

                    sA = scp.tile([128, NT], f32, tag="sA")
                    prod = prodp.tile([128, XW], bf16, tag="prod")
                    for n in range(NT):
                        nc.vector.scalar_tensor_tensor(
                            out=prod,
                            in0=x[:, n, :],
                            scalar=0.0,
                            in1=vbt[:, ibr, :],
                            op0=mybir.AluOpType.bypass,
                            op1=mybir.AluOpType.mult,
                            accum_out=sA[:, n : n + 1],
                        )
                    p_raw = scp.tile([128, NT], bf16, tag="p_raw")
                    nc.scalar.activation(
                        out=p_raw, in_=sA, func=mybir.ActivationFunctionType.Exp
                    )

                    # pp[:, n, :]: col b = p*m0, col 4+b = p*m1, rest 0
                    pp = ppp.tile([128, NT, 8], bf16, tag="pp")
                    nc.vector.memset(pp, 0.0)
                    nc.vector.tensor_mul(pp[:, :, b], p_raw, mkt[:, ibr, 0, b, :])
                    nc.vector.tensor_mul(pp[:, :, 4 + b], p_raw, mkt[:, ibr, 1, b, :])

                    for n in range(NT):
                        first = b == 0 and n == 0
                        last = b == BL - 1 and n == NT - 1
                        nc.tensor.matmul(
                            psAt, pp[:, n, :], x[:, n, 0:D], start=first, stop=last
                        )
                        nc.tensor.matmul(
                            psBt, pp[:, n, :], x[:, n, D:XW], start=first, stop=last
                        )

                # ---- finishing: out = (U0@G0 + U1@G1) / P ------------------
                uall = finp.tile([8, F + 1], f32, tag="uall")
                nc.vector.tensor_copy(uall[:, 0:D], psAt)
                nc.vector.tensor_copy(uall[:, D : F + 1], psBt[:, 0 : KD + 1])

                uallT = finp.tile([128, 7, 8], f32, tag="uallT")
                for k in range(6):
                    trp = psTr.tile([128, 8], f32, tag="trp")
                    nc.tensor.transpose(trp, uall[:, k * 128 : (k + 1) * 128], ident)
                    nc.vector.tensor_copy(uallT[:, k, :], trp)
                trp = psTr.tile([128, 8], f32, tag="trp")
                nc.tensor.transpose(trp[0:1, :], uall[:, F : F + 1], ident)
                nc.vector.tensor_copy(uallT[0:1, 6, :], trp[0:1, :])
                uTb = finp.tile([128, 7, 8], bf16, tag="uTb")
                nc.vector.tensor_copy(uTb, uallT)

                po = psOut.tile([4, D + 1], f32)  # cols 0:D main, col D = P (bank 2)
                g0, g1 = 2 * ibr, 2 * ibr + 1
                for k in range(6):
                    nc.tensor.matmul(
                        po[:, 0:D], uTb[:, k, 0:4], gt[:, g0, k, :],
                        start=(k == 0), stop=False,
                    )
                nc.tensor.matmul(
                    po[:, 0:D], uTb[0:1, 6, 0:4], gt[0:1, g0, 6, :],
                    start=False, stop=False,
                )
                for k in range(6):
                    nc.tensor.matmul(
                        po[:, 0:D], uTb[:, k, 4:8], gt[:, g1, k, :],
                        start=False, stop=False,
                    )
                nc.tensor.matmul(
                    po[:, 0:D], uTb[0:1, 6, 4:8], gt[0:1, g1, 6, :],
                    start=False, stop=True,
                )
                nc.tensor.matmul(
                    po[:, D : D + 1], uTb[0:1, 6, 0:4], ones11, start=True, stop=False
                )
                nc.tensor.matmul(
                    po[:, D : D + 1], uTb[0:1, 6, 4:8], ones11, start=False, stop=True
                )

                rp = finp.tile([4, 1], f32, tag="rp")
                nc.vector.reciprocal(rp, po[:, D : D + 1])
                osb = finp.tile([4, D], f32, tag="osb")
                nc.vector.tensor_scalar_mul(out=osb, in0=po[:, 0:D], scalar1=rp)
                nc.sync.dma_start(out=osrc[:, :], in_=osb)

    nc.compile()
    return nc


def _get_nc():
    if "nc" not in _BUILD_CACHE:
        _BUILD_CACHE["nc"] = _build()
    return _BUILD_CACHE["nc"]


def kernel(**inputs) -> tuple:
    global last_results
    from concourse import mybir
    from concourse.bass_utils import run_bass_kernel_spmd

    f32 = np.float32
    bf16 = np.dtype(mybir.dt.np(mybir.dt.bfloat16))

    K = np.asarray(inputs["K"], dtype=f32)
    front_k1 = np.asarray(inputs["front_k1"], dtype=f32)
    back_K = np.asarray(inputs["back_K"], dtype=f32)
    back_k2 = np.asarray(inputs["back_k2"], dtype=f32)
    Wfk = np.asarray(inputs["Wfk"], dtype=f32)
    bfk = np.asarray(inputs["bfk"], dtype=f32)
    Wbk = np.asarray(inputs["Wbk"], dtype=f32)
    bbk = np.asarray(inputs["bbk"], dtype=f32)
    Wr0 = np.asarray(inputs["Wr0"], dtype=f32)
    Wr1 = np.asarray(inputs["Wr1"], dtype=f32)
    wf_den = np.asarray(inputs["wf_den"], dtype=f32)
    wb_den = np.asarray(inputs["wb_den"], dtype=f32)
    adj_f = np.asarray(inputs["front_sdj_den"], dtype=f32)
    sm_f = np.asarray(inputs["front_s_mask"], dtype=f32)
    adj_b = np.asarray(inputs["back_sdj_den"], dtype=f32)
    sm_b = np.asarray(inputs["back_s_mask"], dtype=f32)
    i = int(np.asarray(inputs["i"]))
    num_utter = int(np.asarray(inputs["num_utter"]))

    # ---- host-folded weights ----------------------------------------------
    v_f = (Wfk.astype(np.float64) @ wf_den[D:].astype(np.float64)).astype(f32)
    v_b = (Wbk.astype(np.float64) @ wb_den[D:].astype(np.float64)).astype(f32)
    A_f = np.vstack([Wfk, bfk[None, :]]).astype(np.float64)
    A_b = np.vstack([Wbk, bbk[None, :]]).astype(np.float64)
    G0_f = (A_f @ Wr0.astype(np.float64)).astype(f32)
    G1_f = (A_f @ Wr1.astype(np.float64)).astype(f32)
    G0_b = (A_b @ Wr0.astype(np.float64)).astype(f32)
    G1_b = (A_b @ Wr1.astype(np.float64)).astype(f32)

    # ---- host-side device layouts -----------------------------------------
    # X = [K | k1 | 1 | 0 0 0] in bf16
    def pack_x(Kv, kf):
        xa = np.zeros((B, N, XW), dtype=bf16)
        xa[:, :, 0:D] = Kv.astype(bf16)
        xa[:, :, D:F] = kf.astype(bf16)
        xa[:, :, F] = np.array(1.0, dtype=bf16)
        return xa

    xall_f = pack_x(K, front_k1)
    xall_b = pack_x(back_K, back_k2)

    # v broadcast [128, 2, XW]
    vbm = np.zeros((128, 2, XW), dtype=bf16)
    vbm[:, 0, 0:F] = v_f.astype(bf16)[None, :]
    vbm[:, 1, 0:F] = v_b.astype(bf16)[None, :]

    # masks [128, 2, 2, B, NT]: mks[p, br, j, b, n] = m_j(b, p*NT + n)
    def mask_pair(adj, sm):
        m0 = (adj * sm).astype(bf16)
        m1 = (adj * (1.0 - sm)).astype(bf16)
        return m0, m1

    m0f, m1f = mask_pair(adj_f, sm_f)
    m0b, m1b = mask_pair(adj_b, sm_b)
    mks = np.empty((128, 2, 2, B, NT), dtype=bf16)
    for j, m in ((0, m0f), (1, m1f)):
        mks[:, 0, j] = m.reshape(B, 128, NT).transpose(1, 0, 2)
    for j, m in ((0, m0b), (1, m1b)):
        mks[:, 1, j] = m.reshape(B, 128, NT).transpose(1, 0, 2)

    # G pack [128, 4, 7, D]: rows 0-767 chunked, row 768 in chunk 6 row 0
    gpk = np.zeros((128, 4, 7, D), dtype=bf16)
    for gi, G in enumerate((G0_f, G1_f, G0_b, G1_b)):
        Gb = G.astype(bf16)
        gpk[:, gi, 0:6, :] = Gb[0:F].reshape(6, 128, D).transpose(1, 0, 2)
        gpk[0, gi, 6, :] = Gb[F]

    nc = _get_nc()

    in_maps = []
    for c in range(NCORES):
        s = slice(c * BL, (c + 1) * BL)
        in_maps.append(
            {
                "x_f": xall_f[s],
                "x_b": xall_b[s],
                "vbm": vbm,
                "mks": np.ascontiguousarray(mks[:, :, :, s, :]),
                "gpk": gpk,
            }
        )

    trace = os.environ.get("KERNEL_TRACE", "0") == "1"
    res = run_bass_kernel_spmd(nc, in_maps, core_ids=list(range(NCORES)), trace=trace)
    last_results = res

    front = np.concatenate([r["out_f"] for r in res.results], axis=0)
    back = np.concatenate([r["out_b"] for r in res.results], axis=0)
    if i == 0:
        front = np.zeros((B, D), dtype=f32)
    if i == num_utter - 1:
        back = np.zeros((B, D), dtype=f32)
    return (front, back)


# revision 14
# speedup vs baseline: 1.5468x; 1.0999x over previous
"""Trainium2 Bass kernel for the DialogGCN GAT-style message-passing layer.

Math notes (why this is much cheaper than the reference graph):
  Kp    = concat(K, kfeat) @ Wk + bk                    (B,N,D)
  alpha = Q@wden[:D] + Kp@wden[D:] + bden               (B,N)
  w     = softmax(alpha - (1-adj)*1e30, axis=N)
  out   = sum_n w * ((Kp@Wr0)*sm + (Kp@Wr1)*(1-sm))

* softmax is invariant to per-row constants, so the Q term, bden and the
  bk@wden[D:] constant all cancel:  w = softmax_n(X_n . v) masked, where
  X = concat(K, kfeat) and v = Wk @ wden[D:]  (folded on host).
* the output is linear in the weighted sums:
    out = (sum_n w*sm*X_n | c0) @ [Wk;bk] @ Wr0 + (sum_n w*(1-sm)*X_n | c1) @ [Wk;bk] @ Wr1
  so G0 = [Wk;bk]@Wr0 and G1 = [Wk;bk]@Wr1 are folded on host (769x512 each).
* v is folded INTO the streamed tensor on host: X' = X * v (columnwise) and
  G' = G / v (rowwise) — exact algebra. The device then computes
    s_n = rowsum(X'_n) ; p_n = exp(s_n) ; U0 = sum p*m0*[X'|1] ; U1 = ...
  followed by a tiny projection (U0@G0' + U1@G1') / P, with m0 = adj*sm,
  m1 = adj*(1-sm), P = row 768 of U (the ones column of X'; the ones column
  also shifts every score by +1, which softmax cancels).

Device-side layout tricks:
* X' is uploaded as ONE bf16 tensor [BL, N, 772] = [K*v | k1*v | 1.0 | 0 0 0]
  (772 keeps every 128-token chunk 4B/8B aligned). This halves HBM traffic,
  turns the score pass into a single DVE tensor_reduce per batch (row sums,
  eligible for packed 2x/4x modes), and the ones column makes the softmax
  denominator fall out of the same PE accumulation that computes U.
* masks and the G projection matrices are pre-swizzled to their SBUF layouts
  on host and uploaded bf16, so every DMA is a dtype-preserving HWDGE
  transfer with contiguous per-partition descriptors.

Sharding: pure data parallel over batch B=32 across 8 cores (4 rows each).
"""

import os
import sys

import numpy as np

for _p in ("/opt/trn_rl_repo", "/root/.axon_site/_ro/trn_rl_repo"):
    if os.path.isdir(_p) and _p not in sys.path:
        sys.path.insert(0, _p)

B, N, D, KD = 32, 2048, 512, 256
F = D + KD  # 768
XW = F + 4  # 772: [K | k1 | 1 | 0 0 0] -- pad keeps chunk offsets 8B aligned
NCORES = 8
BL = B // NCORES  # 4 batch rows per core
NT = 16  # free-dim token tiles per batch (N = 128 * NT)

_BUILD_CACHE = {}
last_results = None  # BassKernelResults of the most recent run (for test.py)


def _build():
    """Trace the Bass program (same NEFF runs SPMD on all 8 cores)."""
    import concourse.bass as bass
    import concourse.tile as tile
    from concourse import bacc, mybir
    from concourse.masks import make_identity

    f32 = mybir.dt.float32
    bf16 = mybir.dt.bfloat16

    nc = bacc.Bacc()

    # ---- DRAM I/O ----------------------------------------------------------
    x_f = nc.dram_tensor("x_f", [BL, N, XW], bf16, kind="ExternalInput")
    x_b = nc.dram_tensor("x_b", [BL, N, XW], bf16, kind="ExternalInput")
    # masks pre-swizzled: [partition, branch, m0/m1, batch, n]
    mks = nc.dram_tensor("mks", [128, 2, 2, BL, NT], bf16, kind="ExternalInput")
    # G matrices pre-swizzled: [partition, (G0_f,G1_f,G0_b,G1_b), chunk, D]
    gpk = nc.dram_tensor("gpk", [128, 4, 6, D], bf16, kind="ExternalInput")
    # row 768 of each G (the bias row)
    g768 = nc.dram_tensor("g768", [1, 4, D], bf16, kind="ExternalInput")
    out_f = nc.dram_tensor("out_f", [BL, D], f32, kind="ExternalOutput")
    out_b = nc.dram_tensor("out_b", [BL, D], f32, kind="ExternalOutput")

    with tile.TileContext(nc) as tc:
        with (
            tc.tile_pool(name="singles", bufs=1) as singles,
            tc.tile_pool(name="xp", bufs=4) as xp,
            tc.tile_pool(name="scp", bufs=2) as scp,
            tc.tile_pool(name="ppp", bufs=3) as ppp,
            tc.tile_pool(name="finp", bufs=2) as finp,
            tc.tile_pool(name="psA", bufs=2, space="PSUM") as psA,
            tc.tile_pool(name="psB", bufs=2, space="PSUM") as psB,
            tc.tile_pool(name="psTr", bufs=2, space="PSUM") as psTr,
            tc.tile_pool(name="psOut", bufs=1, space="PSUM") as psOut,
        ):
            # ---- one-time setup -------------------------------------------
            ident = singles.tile([8, 8], f32)
            make_identity(nc, ident)
            ones11 = singles.tile([1, 1], bf16)
            nc.vector.memset(ones11, 1.0)
            negone = singles.tile([128, 1], f32)
            nc.vector.memset(negone, -1.0)

            mkt = singles.tile([128, 2, 2, BL, NT], bf16)
            nc.scalar.dma_start(out=mkt, in_=mks[:, :, :, :, :])
            gt = singles.tile([128, 4, 6, D], bf16)
            nc.scalar.dma_start(out=gt, in_=gpk[:, :, :, :])
            g768t = singles.tile([1, 4, D], bf16)
            nc.scalar.dma_start(out=g768t, in_=g768[:, :, :])

            # ---- streaming + finishing per branch -------------------------
            for ibr, (xsrc, osrc) in enumerate(((x_f, out_f), (x_b, out_b))):
                psAt = psA.tile([8, D], f32)       # rows 0-3: U0(b) K-part, 4-7: U1(b)
                psBt = psB.tile([8, KD + 4], f32)  # cols 0:KD k1-part, col KD = P, pad

                for b in range(BL):
                    x = xp.tile([128, NT, XW], bf16, tag="x")
                    nc.sync.dma_start(
                        out=x, in_=xsrc[b].rearrange("(p n) d -> p n d", n=NT)
                    )

                    sA = scp.tile([128, NT], bf16, tag="sA")
                    with nc.allow_low_precision(
                        reason="DVE ALUs accumulate fp32; single bf16 round on write"
                    ):
                        nc.vector.tensor_reduce(
                            out=sA,
                            in_=x[:, :, :],
                            axis=mybir.AxisListType.X,
                            op=mybir.AluOpType.add,
                        )
                    p_raw = scp.tile([128, NT], bf16, tag="p_raw")
                    # bias=-1 removes the constant from the ones column
                    nc.scalar.activation(
                        out=p_raw,
                        in_=sA,
                        func=mybir.ActivationFunctionType.Exp,
                        bias=negone,
                    )

                    # pp[:, n, :]: col b = p*m0, col 4+b = p*m1, rest 0
                    pp = ppp.tile([128, NT, 8], bf16, tag="pp")
                    nc.vector.memset(pp, 0.0)
                    nc.vector.tensor_mul(pp[:, :, b], p_raw, mkt[:, ibr, 0, b, :])
                    nc.vector.tensor_mul(pp[:, :, 4 + b], p_raw, mkt[:, ibr, 1, b, :])

                    for n in range(NT):
                        first = b == 0 and n == 0
                        last = b == BL - 1 and n == NT - 1
                        nc.tensor.matmul(
                            psAt, pp[:, n, :], x[:, n, 0:D], start=first, stop=last
                        )
                        nc.tensor.matmul(
                            psBt, pp[:, n, :], x[:, n, D:XW], start=first, stop=last
                        )

                # ---- finishing: out = (U0@G0 + U1@G1) / P ------------------
                uall = finp.tile([8, F + 1], f32, tag="uall")
                nc.vector.tensor_copy(uall[:, 0:D], psAt)
                nc.vector.tensor_copy(uall[:, D : F + 1], psBt[:, 0 : KD + 1])

                uallT = finp.tile([128, 7, 8], f32, tag="uallT")
                for k in range(6):
                    trp = psTr.tile([128, 8], f32, tag="trp")
                    nc.tensor.transpose(trp, uall[:, k * 128 : (k + 1) * 128], ident)
                    nc.vector.tensor_copy(uallT[:, k, :], trp)
                trp = psTr.tile([128, 8], f32, tag="trp")
                nc.tensor.transpose(trp[0:1, :], uall[:, F : F + 1], ident)
                nc.vector.tensor_copy(uallT[0:1, 6, :], trp[0:1, :])
                uTb = finp.tile([128, 7, 8], bf16, tag="uTb")
                nc.vector.tensor_copy(uTb, uallT)

                po = psOut.tile([4, D + 1], f32)  # cols 0:D main, col D = P (bank 2)
                g0, g1 = 2 * ibr, 2 * ibr + 1
                for k in range(6):
                    nc.tensor.matmul(
                        po[:, 0:D], uTb[:, k, 0:4], gt[:, g0, k, :],
                        start=(k == 0), stop=False,
                    )
                nc.tensor.matmul(
                    po[:, 0:D], uTb[0:1, 6, 0:4], g768t[0:1, g0, :],
                    start=False, stop=False,
                )
                for k in range(6):
                    nc.tensor.matmul(
                        po[:, 0:D], uTb[:, k, 4:8], gt[:, g1, k, :],
                        start=False, stop=False,
                    )
                nc.tensor.matmul(
                    po[:, 0:D], uTb[0:1, 6, 4:8], g768t[0:1, g1, :],
                    start=False, stop=True,
                )
                nc.tensor.matmul(
                    po[:, D : D + 1], uTb[0:1, 6, 0:4], ones11, start=True, stop=False
                )
                nc.tensor.matmul(
                    po[:, D : D + 1], uTb[0:1, 6, 4:8], ones11, start=False, stop=True
                )

                rp = finp.tile([4, 1], f32, tag="rp")
                nc.vector.reciprocal(rp, po[:, D : D + 1])
                osb = finp.tile([4, D], f32, tag="osb")
                nc.vector.tensor_scalar_mul(out=osb, in0=po[:, 0:D], scalar1=rp)
                nc.sync.dma_start(out=osrc[:, :], in_=osb)

    nc.compile()
    return nc


def _get_nc():
    if "nc" not in _BUILD_CACHE:
        _BUILD_CACHE["nc"] = _build()
    return _BUILD_CACHE["nc"]


def kernel(**inputs) -> tuple:
    global last_results
    from concourse import mybir
    from concourse.bass_utils import run_bass_kernel_spmd

    f32 = np.float32
    bf16 = np.dtype(mybir.dt.np(mybir.dt.bfloat16))

    K = np.asarray(inputs["K"], dtype=f32)
    front_k1 = np.asarray(inputs["front_k1"], dtype=f32)
    back_K = np.asarray(inputs["back_K"], dtype=f32)
    back_k2 = np.asarray(inputs["back_k2"], dtype=f32)
    Wfk = np.asarray(inputs["Wfk"], dtype=f32)
    bfk = np.asarray(inputs["bfk"], dtype=f32)
    Wbk = np.asarray(inputs["Wbk"], dtype=f32)
    bbk = np.asarray(inputs["bbk"], dtype=f32)
    Wr0 = np.asarray(inputs["Wr0"], dtype=f32)
    Wr1 = np.asarray(inputs["Wr1"], dtype=f32)
    wf_den = np.asarray(inputs["wf_den"], dtype=f32)
    wb_den = np.asarray(inputs["wb_den"], dtype=f32)
    adj_f = np.asarray(inputs["front_sdj_den"], dtype=f32)
    sm_f = np.asarray(inputs["front_s_mask"], dtype=f32)
    adj_b = np.asarray(inputs["back_sdj_den"], dtype=f32)
    sm_b = np.asarray(inputs["back_s_mask"], dtype=f32)
    i = int(np.asarray(inputs["i"]))
    num_utter = int(np.asarray(inputs["num_utter"]))

    # ---- host-folded weights ----------------------------------------------
    v_f = (Wfk.astype(np.float64) @ wf_den[D:].astype(np.float64)).astype(f32)
    v_b = (Wbk.astype(np.float64) @ wb_den[D:].astype(np.float64)).astype(f32)
    A_f = np.vstack([Wfk, bfk[None, :]]).astype(np.float64)
    A_b = np.vstack([Wbk, bbk[None, :]]).astype(np.float64)
    G0_f = (A_f @ Wr0.astype(np.float64)).astype(f32)
    G1_f = (A_f @ Wr1.astype(np.float64)).astype(f32)
    G0_b = (A_b @ Wr0.astype(np.float64)).astype(f32)
    G1_b = (A_b @ Wr1.astype(np.float64)).astype(f32)

    # ---- host-side device layouts -----------------------------------------
    # clamp v away from 0 so the X*v / G/v fold is always well-conditioned
    def clamp(v):
        tiny = np.float32(1e-12)
        return np.where(np.abs(v) < tiny, np.where(v >= 0, tiny, -tiny), v)

    vs_f = clamp(v_f)
    vs_b = clamp(v_b)

    # X' = [K*v | k1*v | 1 | 0 0 0] in bf16
    def pack_x(Kv, kf, vs):
        xa = np.zeros((B, N, XW), dtype=bf16)
        xa[:, :, 0:D] = (Kv * vs[0:D]).astype(bf16)
        xa[:, :, D:F] = (kf * vs[D:F]).astype(bf16)
        xa[:, :, F] = np.array(1.0, dtype=bf16)
        return xa

    xall_f = pack_x(K, front_k1, vs_f)
    xall_b = pack_x(back_K, back_k2, vs_b)

    # masks [128, 2, 2, B, NT]: mks[p, br, j, b, n] = m_j(b, p*NT + n)
    def mask_pair(adj, sm):
        m0 = (adj * sm).astype(bf16)
        m1 = (adj * (1.0 - sm)).astype(bf16)
        return m0, m1

    m0f, m1f = mask_pair(adj_f, sm_f)
    m0b, m1b = mask_pair(adj_b, sm_b)
    mks = np.empty((128, 2, 2, B, NT), dtype=bf16)
    for j, m in ((0, m0f), (1, m1f)):
        mks[:, 0, j] = m.reshape(B, 128, NT).transpose(1, 0, 2)
    for j, m in ((0, m0b), (1, m1b)):
        mks[:, 1, j] = m.reshape(B, 128, NT).transpose(1, 0, 2)

    # G' pack [128, 4, 6, D]: rows 0-767 divided by v, chunked; row 768 apart
    gpk = np.empty((128, 4, 6, D), dtype=bf16)
    g768 = np.empty((1, 4, D), dtype=bf16)
    for gi, (G, vs) in enumerate(
        ((G0_f, vs_f), (G1_f, vs_f), (G0_b, vs_b), (G1_b, vs_b))
    ):
        Gp = (G[0:F] / vs[:, None]).astype(bf16)
        gpk[:, gi] = Gp.reshape(6, 128, D).transpose(1, 0, 2)
        g768[0, gi] = G[F].astype(bf16)

    nc = _get_nc()

    in_maps = []
    for c in range(NCORES):
        s = slice(c * BL, (c + 1) * BL)
        in_maps.append(
            {
                "x_f": xall_f[s],
                "x_b": xall_b[s],
                "mks": np.ascontiguousarray(mks[:, :, :, s, :]),
                "gpk": gpk,
                "g768": g768,
            }
        )

    trace = os.environ.get("KERNEL_TRACE", "0") == "1"
    res = run_bass_kernel_spmd(nc, in_maps, core_ids=list(range(NCORES)), trace=trace)
    last_results = res

    front = np.concatenate([r["out_f"] for r in res.results], axis=0)
    back = np.concatenate([r["out_b"] for r in res.results], axis=0)
    if i == 0:
        front = np.zeros((B, D), dtype=f32)
    if i == num_utter - 1:
        back = np.zeros((B, D), dtype=f32)
    return (front, back)


# revision 18
# speedup vs baseline: 1.9036x; 1.2307x over previous
"""Trainium2 Bass kernel for the DialogGCN GAT-style message-passing layer.

Math notes (why this is much cheaper than the reference graph):
  Kp    = concat(K, kfeat) @ Wk + bk                    (B,N,D)
  alpha = Q@wden[:D] + Kp@wden[D:] + bden               (B,N)
  w     = softmax(alpha - (1-adj)*1e30, axis=N)
  out   = sum_n w * ((Kp@Wr0)*sm + (Kp@Wr1)*(1-sm))

* softmax is invariant to per-row constants, so the Q term, bden and the
  bk@wden[D:] constant all cancel:  w = softmax_n(X_n . v) masked, where
  X = concat(K, kfeat) and v = Wk @ wden[D:]  (folded on host).
* the output is linear in the weighted sums:
    out = (sum_n w*sm*X_n | c0) @ [Wk;bk] @ Wr0 + (sum_n w*(1-sm)*X_n | c1) @ [Wk;bk] @ Wr1
  so G0 = [Wk;bk]@Wr0 and G1 = [Wk;bk]@Wr1 are folded on host (769x512 each).
* v is folded INTO the streamed tensor on host: X' = X * v (columnwise) and
  G' = G / v (rowwise) — exact algebra. The device then computes
    s_n = rowsum(X'_n) ; p_n = exp(s_n) ; U0 = sum p*m0*[X'|1] ; U1 = ...
  followed by a tiny projection (U0@G0' + U1@G1') / P, with m0 = adj*sm,
  m1 = adj*(1-sm), P = row 768 of U (the ones column of X'; the ones column
  also shifts every score by +1, which softmax cancels).

Device-side layout tricks:
* X' is uploaded as ONE bf16 tensor [BL, N, 772] = [K*v | k1*v | 1.0 | 0 0 0]
  (772 keeps every 128-token chunk 4B/8B aligned). This halves HBM traffic,
  turns the score pass into a single DVE tensor_reduce per batch (row sums,
  eligible for packed 2x/4x modes), and the ones column makes the softmax
  denominator fall out of the same PE accumulation that computes U.
* masks and the G projection matrices are pre-swizzled to their SBUF layouts
  on host and uploaded bf16, so every DMA is a dtype-preserving HWDGE
  transfer with contiguous per-partition descriptors.

Sharding: pure data parallel over batch B=32 across 8 cores (4 rows each).
"""

import os
import sys

import numpy as np

for _p in ("/opt/trn_rl_repo", "/root/.axon_site/_ro/trn_rl_repo"):
    if os.path.isdir(_p) and _p not in sys.path:
        sys.path.insert(0, _p)

B, N, D, KD = 32, 2048, 512, 256
F = D + KD  # 768
XW = F + 4  # 772: [K | k1 | 1 | 0 0 0] -- pad keeps chunk offsets 8B aligned
NCORES = 8
BL = B // NCORES  # 4 batch rows per core
NT = 16  # free-dim token tiles per batch (N = 128 * NT)

_BUILD_CACHE = {}
last_results = None  # BassKernelResults of the most recent run (for test.py)


def _build():
    """Trace the Bass program (same NEFF runs SPMD on all 8 cores)."""
    import concourse.bass as bass
    import concourse.tile as tile
    from concourse import bacc, mybir
    from concourse.masks import make_identity

    f32 = mybir.dt.float32
    bf16 = mybir.dt.bfloat16

    nc = bacc.Bacc()

    # ---- DRAM I/O ----------------------------------------------------------
    x_f = nc.dram_tensor("x_f", [BL, N, XW], bf16, kind="ExternalInput")
    x_b = nc.dram_tensor("x_b", [BL, N, XW], bf16, kind="ExternalInput")
    # masks pre-swizzled: [partition, branch, m0/m1, batch, n]
    mks = nc.dram_tensor("mks", [128, 2, 2, BL, NT], bf16, kind="ExternalInput")
    # G matrices pre-swizzled: [partition, (G0_f,G1_f,G0_b,G1_b), chunk, D]
    gpk = nc.dram_tensor("gpk", [128, 4, 6, D], bf16, kind="ExternalInput")
    # row 768 of each G (the bias row)
    g768 = nc.dram_tensor("g768", [1, 4, D], bf16, kind="ExternalInput")
    out_f = nc.dram_tensor("out_f", [BL, D], f32, kind="ExternalOutput")
    out_b = nc.dram_tensor("out_b", [BL, D], f32, kind="ExternalOutput")

    with tile.TileContext(nc) as tc:
        with (
            tc.tile_pool(name="singles", bufs=1) as singles,
            tc.tile_pool(name="xp", bufs=5) as xp,
            tc.tile_pool(name="scp", bufs=2) as scp,
            tc.tile_pool(name="ppp", bufs=4) as ppp,
            tc.tile_pool(name="finp", bufs=2) as finp,
            tc.tile_pool(name="psA", bufs=2, space="PSUM") as psA,
            tc.tile_pool(name="psB", bufs=2, space="PSUM") as psB,
            tc.tile_pool(name="psTr", bufs=2, space="PSUM") as psTr,
            tc.tile_pool(name="psOut", bufs=1, space="PSUM") as psOut,
        ):
            # ---- X loads: first 5 upfront so SP starts streaming at once;
            # the rest are emitted as compute iterations free their buffers
            NB = 2 * BL
            xsrcs = (x_f, x_b)
            xtiles = {}

            def emit_xdma(g):
                ibr, b = divmod(g, BL)
                x = xp.tile([128, NT, XW], bf16, tag="x")
                nc.sync.dma_start(
                    out=x, in_=xsrcs[ibr][b].rearrange("(p n) d -> p n d", n=NT)
                )
                xtiles[g] = x

            for g in range(5):
                emit_xdma(g)

            # ---- one-time setup -------------------------------------------
            ident = singles.tile([8, 8], f32)
            make_identity(nc, ident)
            ones11 = singles.tile([1, 1], bf16)
            nc.vector.memset(ones11, 1.0)
            negone = singles.tile([128, 1], f32)
            nc.vector.memset(negone, -1.0)

            mkt = singles.tile([128, 2, 2, BL, NT], bf16)
            nc.scalar.dma_start(out=mkt, in_=mks[:, :, :, :, :])
            gt = singles.tile([128, 4, 6, D], bf16)
            nc.scalar.dma_start(out=gt, in_=gpk[:, :, :, :])
            g768t = singles.tile([1, 4, D], bf16)
            nc.scalar.dma_start(out=g768t, in_=g768[:, :, :])

            # ---- streaming + finishing per branch -------------------------
            for ibr, (xsrc, osrc) in enumerate(((x_f, out_f), (x_b, out_b))):
                psAt = psA.tile([8, D], f32)       # rows 0-3: U0(b) K-part, 4-7: U1(b)
                psBt = psB.tile([8, KD + 4], f32)  # cols 0:KD k1-part, col KD = P, pad

                for b in range(BL):
                    g = ibr * BL + b
                    x = xtiles[g]
                    NH = NT // 2

                    # scores, split across the idle ACT engine (chunks 0-7,
                    # one accum per chunk) and the DVE (chunks 8-15, one
                    # strided reduce); both halves then exp on ACT
                    sA0 = scp.tile([128, NH], f32, tag="sA0")
                    scr = scp.tile([128, XW], bf16, tag="scr")
                    for n in range(NH):
                        nc.scalar.activation(
                            out=scr,
                            in_=x[:, n, :],
                            func=mybir.ActivationFunctionType.Copy,
                            accum_out=sA0[:, n : n + 1],
                        )
                    sA1 = scp.tile([128, NH], f32, tag="sA1")
                    nc.vector.tensor_reduce(
                        out=sA1,
                        in_=x[:, NH:NT, :],
                        axis=mybir.AxisListType.X,
                        op=mybir.AluOpType.add,
                    )
                    p0 = scp.tile([128, NH], bf16, tag="p0")
                    p1 = scp.tile([128, NH], bf16, tag="p1")
                    # bias=-1 removes the constant from the ones column
                    nc.scalar.activation(
                        out=p0, in_=sA0, func=mybir.ActivationFunctionType.Exp,
                        bias=negone,
                    )
                    nc.scalar.activation(
                        out=p1, in_=sA1, func=mybir.ActivationFunctionType.Exp,
                        bias=negone,
                    )

                    # pp[:, n, :]: col b = p*m0, col 4+b = p*m1, rest 0
                    pps = []
                    for h, ph in ((0, p0), (1, p1)):
                        pp = ppp.tile([128, NH, 8], bf16, tag="pp")
                        nc.vector.memset(pp, 0.0)
                        lo, hi = h * NH, (h + 1) * NH
                        nc.vector.tensor_mul(
                            pp[:, :, b], ph, mkt[:, ibr, 0, b, lo:hi]
                        )
                        nc.vector.tensor_mul(
                            pp[:, :, 4 + b], ph, mkt[:, ibr, 1, b, lo:hi]
                        )
                        pps.append(pp)

                    for n in range(NT):
                        first = b == 0 and n == 0
                        last = b == BL - 1 and n == NT - 1
                        ppt = pps[n // NH]
                        nn = n % NH
                        nc.tensor.matmul(
                            psAt, ppt[:, nn, :], x[:, n, 0:D], start=first, stop=last
                        )
                        nc.tensor.matmul(
                            psBt, ppt[:, nn, :], x[:, n, D:XW], start=first, stop=last
                        )

                    if g + 5 < NB:
                        emit_xdma(g + 5)

                # ---- finishing: out = (U0@G0 + U1@G1) / P ------------------
                uall = finp.tile([8, F + 1], f32, tag="uall")
                nc.vector.tensor_copy(uall[:, 0:D], psAt)
                nc.vector.tensor_copy(uall[:, D : F + 1], psBt[:, 0 : KD + 1])

                uallT = finp.tile([128, 7, 8], f32, tag="uallT")
                for k in range(6):
                    trp = psTr.tile([128, 8], f32, tag="trp")
                    nc.tensor.transpose(trp, uall[:, k * 128 : (k + 1) * 128], ident)
                    nc.vector.tensor_copy(uallT[:, k, :], trp)
                trp = psTr.tile([128, 8], f32, tag="trp")
                nc.tensor.transpose(trp[0:1, :], uall[:, F : F + 1], ident)
                nc.vector.tensor_copy(uallT[0:1, 6, :], trp[0:1, :])
                uTb = finp.tile([128, 7, 8], bf16, tag="uTb")
                nc.vector.tensor_copy(uTb, uallT)

                po = psOut.tile([4, D + 1], f32)  # cols 0:D main, col D = P (bank 2)
                g0, g1 = 2 * ibr, 2 * ibr + 1
                for k in range(6):
                    nc.tensor.matmul(
                        po[:, 0:D], uTb[:, k, 0:4], gt[:, g0, k, :],
                        start=(k == 0), stop=False,
                    )
                nc.tensor.matmul(
                    po[:, 0:D], uTb[0:1, 6, 0:4], g768t[0:1, g0, :],
                    start=False, stop=False,
                )
                for k in range(6):
                    nc.tensor.matmul(
                        po[:, 0:D], uTb[:, k, 4:8], gt[:, g1, k, :],
                        start=False, stop=False,
                    )
                nc.tensor.matmul(
                    po[:, 0:D], uTb[0:1, 6, 4:8], g768t[0:1, g1, :],
                    start=False, stop=True,
                )
                nc.tensor.matmul(
                    po[:, D : D + 1], uTb[0:1, 6, 0:4], ones11, start=True, stop=False
                )
                nc.tensor.matmul(
                    po[:, D : D + 1], uTb[0:1, 6, 4:8], ones11, start=False, stop=True
                )

                rp = finp.tile([4, 1], f32, tag="rp")
                nc.vector.reciprocal(rp, po[:, D : D + 1])
                osb = finp.tile([4, D], f32, tag="osb")
                nc.vector.tensor_scalar_mul(out=osb, in0=po[:, 0:D], scalar1=rp)
                nc.sync.dma_start(out=osrc[:, :], in_=osb)

    nc.compile()
    return nc


def _get_nc():
    if "nc" not in _BUILD_CACHE:
        _BUILD_CACHE["nc"] = _build()
    return _BUILD_CACHE["nc"]


def kernel(**inputs) -> tuple:
    global last_results
    from concourse import mybir
    from concourse.bass_utils import run_bass_kernel_spmd

    f32 = np.float32
    bf16 = np.dtype(mybir.dt.np(mybir.dt.bfloat16))

    K = np.asarray(inputs["K"], dtype=f32)
    front_k1 = np.asarray(inputs["front_k1"], dtype=f32)
    back_K = np.asarray(inputs["back_K"], dtype=f32)
    back_k2 = np.asarray(inputs["back_k2"], dtype=f32)
    Wfk = np.asarray(inputs["Wfk"], dtype=f32)
    bfk = np.asarray(inputs["bfk"], dtype=f32)
    Wbk = np.asarray(inputs["Wbk"], dtype=f32)
    bbk = np.asarray(inputs["bbk"], dtype=f32)
    Wr0 = np.asarray(inputs["Wr0"], dtype=f32)
    Wr1 = np.asarray(inputs["Wr1"], dtype=f32)
    wf_den = np.asarray(inputs["wf_den"], dtype=f32)
    wb_den = np.asarray(inputs["wb_den"], dtype=f32)
    adj_f = np.asarray(inputs["front_sdj_den"], dtype=f32)
    sm_f = np.asarray(inputs["front_s_mask"], dtype=f32)
    adj_b = np.asarray(inputs["back_sdj_den"], dtype=f32)
    sm_b = np.asarray(inputs["back_s_mask"], dtype=f32)
    i = int(np.asarray(inputs["i"]))
    num_utter = int(np.asarray(inputs["num_utter"]))

    # ---- host-folded weights ----------------------------------------------
    v_f = (Wfk.astype(np.float64) @ wf_den[D:].astype(np.float64)).astype(f32)
    v_b = (Wbk.astype(np.float64) @ wb_den[D:].astype(np.float64)).astype(f32)
    A_f = np.vstack([Wfk, bfk[None, :]]).astype(np.float64)
    A_b = np.vstack([Wbk, bbk[None, :]]).astype(np.float64)
    G0_f = (A_f @ Wr0.astype(np.float64)).astype(f32)
    G1_f = (A_f @ Wr1.astype(np.float64)).astype(f32)
    G0_b = (A_b @ Wr0.astype(np.float64)).astype(f32)
    G1_b = (A_b @ Wr1.astype(np.float64)).astype(f32)

    # ---- host-side device layouts -----------------------------------------
    # clamp v away from 0 so the X*v / G/v fold is always well-conditioned
    def clamp(v):
        tiny = np.float32(1e-12)
        return np.where(np.abs(v) < tiny, np.where(v >= 0, tiny, -tiny), v)

    vs_f = clamp(v_f)
    vs_b = clamp(v_b)

    # X' = [K*v | k1*v | 1 | 0 0 0] in bf16
    def pack_x(Kv, kf, vs):
        xa = np.zeros((B, N, XW), dtype=bf16)
        xa[:, :, 0:D] = (Kv * vs[0:D]).astype(bf16)
        xa[:, :, D:F] = (kf * vs[D:F]).astype(bf16)
        xa[:, :, F] = np.array(1.0, dtype=bf16)
        return xa

    xall_f = pack_x(K, front_k1, vs_f)
    xall_b = pack_x(back_K, back_k2, vs_b)

    # masks [128, 2, 2, B, NT]: mks[p, br, j, b, n] = m_j(b, p*NT + n)
    def mask_pair(adj, sm):
        m0 = (adj * sm).astype(bf16)
        m1 = (adj * (1.0 - sm)).astype(bf16)
        return m0, m1

    m0f, m1f = mask_pair(adj_f, sm_f)
    m0b, m1b = mask_pair(adj_b, sm_b)
    mks = np.empty((128, 2, 2, B, NT), dtype=bf16)
    for j, m in ((0, m0f), (1, m1f)):
        mks[:, 0, j] = m.reshape(B, 128, NT).transpose(1, 0, 2)
    for j, m in ((0, m0b), (1, m1b)):
        mks[:, 1, j] = m.reshape(B, 128, NT).transpose(1, 0, 2)

    # G' pack [128, 4, 6, D]: rows 0-767 divided by v, chunked; row 768 apart
    gpk = np.empty((128, 4, 6, D), dtype=bf16)
    g768 = np.empty((1, 4, D), dtype=bf16)
    for gi, (G, vs) in enumerate(
        ((G0_f, vs_f), (G1_f, vs_f), (G0_b, vs_b), (G1_b, vs_b))
    ):
        Gp = (G[0:F] / vs[:, None]).astype(bf16)
        gpk[:, gi] = Gp.reshape(6, 128, D).transpose(1, 0, 2)
        g768[0, gi] = G[F].astype(bf16)

    nc = _get_nc()

    in_maps = []
    for c in range(NCORES):
        s = slice(c * BL, (c + 1) * BL)
        in_maps.append(
            {
                "x_f": xall_f[s],
                "x_b": xall_b[s],
                "mks": np.ascontiguousarray(mks[:, :, :, s, :]),
                "gpk": gpk,
                "g768": g768,
            }
        )

    trace = os.environ.get("KERNEL_TRACE", "0") == "1"
    res = run_bass_kernel_spmd(nc, in_maps, core_ids=list(range(NCORES)), trace=trace)
    last_results = res

    front = np.concatenate([r["out_f"] for r in res.results], axis=0)
    back = np.concatenate([r["out_b"] for r in res.results], axis=0)
    if i == 0:
        front = np.zeros((B, D), dtype=f32)
    if i == num_utter - 1:
        back = np.zeros((B, D), dtype=f32)
    return (front, back)
